# revision 14
# baseline (speedup 1.0000x reference)
"""BiLSTM-CRF Trainium2 kernel (8 NeuronCores, SPMD).

Sharding: data-parallel over batch (B=64 -> 16 per core); cores 0-3 run the
forward LSTM direction, cores 4-7 the backward direction for the same batch
slices. All 8 cores execute an IDENTICAL instruction stream; every fwd/bwd
asymmetry lives in per-core host-prepared data (time-reversed inputs,
transposed transition matrix, reordered gather indices).

Launch 1 (per core):
  A: xW = [embeds;1] @ [wih;bias]   input projection for all 256 steps
  B: 256-step LSTM recurrence (PE: whh matmuls accumulating onto xW; ACT:
     tanh/sigmoid; DVE: cell/hidden updates); h kept in SBUF as (hid, token)
  C: tag-space partial projection ownP[token, tag] = h(token)^T @ W_half

Host: reorders partial rows into each core's Viterbi scan order (the
reference's torch .view(B,S,H) reinterpretation permutes which LSTM output
each (b, s) consumes: token = (4b + s//64, s%64)). Pure gather, no math.

Launch 2 (per core):
  sfeats = sfA + sfB (fwd+bwd partials); 256-step max-plus Viterbi value
  scan (fwd scan on cores 0-3, suffix scan on cores 4-7 via transposed
  transitions + reversed feats -- same code); pair AllGather of pre/uv;
  path[s] = argmax_j(pre_s[j] + uv_s[j]) fully parallel; score = tot at s=0.
"""

import sys

if "/opt/trn_rl_repo" not in sys.path:
    sys.path.insert(0, "/opt/trn_rl_repo")

import numpy as np

import concourse.bass as bass
import concourse.mybir as mybir
from concourse.tile import TileContext
from concourse.bass_utils import run_bass_kernel_spmd

V, E, H, H2, T, B, S = 50000, 300, 512, 256, 10, 64, 256
START_IDX, STOP_IDX = 8, 9
NCORES = 8
BL = B // 4          # 16 batch elements per core
NTOK = S * BL        # 4096 tokens per core
EA = E + 1           # embedding dim + constant-1 bias row
G = 4 * H2           # 1024 gate rows
NEG = -1000000.0

# LSTM compute dtype: float32 (exact-ish) or bfloat16 (fast)
LSTM_DT = mybir.dt.float32

F32 = mybir.dt.float32
AX = mybir.AxisListType
OP = mybir.AluOpType
ACTF = mybir.ActivationFunctionType

_wid = [0]


def _split_multiwaits(nc):
    """This container's walrus accepts only ONE sync wait per instruction;
    hoist extra waits onto same-engine NoOps placed just before."""
    for f in nc.m.functions:
        for bb in f.blocks:
            old = bb.instructions
            new = []
            changed = False
            for ins in old:
                si = getattr(ins, "sync_info", None)
                if si is not None and si.on_wait and len(si.on_wait) > 1:
                    waits = list(si.on_wait)
                    for w in waits[:-1]:
                        _wid[0] += 1
                        nop = mybir.InstNoOp(name=f"WSPLIT-{_wid[0]}", ins=[], outs=[])
                        nop.engine = ins.engine
                        nop.sync_info = mybir.SyncInfo(on_wait=[w], on_update=[])
                        new.append(nop)
                    ins.sync_info = mybir.SyncInfo(
                        on_wait=[waits[-1]], on_update=list(si.on_update)
                    )
                    changed = True
                new.append(ins)
            if changed:
                bb.instructions = new


def build_l1():
    nc = bass.Bass()

    embT = nc.dram_tensor("embT", [EA, NTOK], F32, kind="ExternalInput")
    wihT = nc.dram_tensor("wihT", [EA, G], F32, kind="ExternalInput")
    whhT = nc.dram_tensor("whhT", [H2, G], LSTM_DT, kind="ExternalInput")
    h0T = nc.dram_tensor("h0T", [H2, BL], LSTM_DT, kind="ExternalInput")
    c0T = nc.dram_tensor("c0T", [H2, BL], F32, kind="ExternalInput")
    WoT = nc.dram_tensor("WoT", [H2, T], LSTM_DT, kind="ExternalInput")
    ownP_o = nc.dram_tensor("ownP_o", [128, 32 * T], F32, kind="ExternalOutput")

    NCH = 16              # phase-A token chunks
    CHT = NTOK // NCH     # 256 tokens per chunk

    with TileContext(nc) as tc:
        with tc.tile_pool(name="const", bufs=1) as constp, \
             tc.tile_pool(name="psum", bufs=2, space="PSUM") as psp:

            WoT_sb = []
            for k in range(2):
                w = constp.tile([128, T], LSTM_DT, name=f"WoT{k}")
                nc.sync.dma_start(out=w, in_=WoT[128 * k:128 * (k + 1), :])
                WoT_sb.append(w)

            with tc.tile_pool(name="lstm", bufs=1) as lstmp:
                whh_sb = []
                for k in range(2):
                    w = lstmp.tile([128, G], LSTM_DT, name=f"whh{k}")
                    nc.sync.dma_start(out=w, in_=whhT[128 * k:128 * (k + 1), :])
                    whh_sb.append(w)
                # h_all[k]: hidden half k, (128, 16*(S+1)); col block t holds
                # h_{t-1}; block 0 = h0
                h_all = []
                for k in range(2):
                    h = lstmp.tile([128, BL * (S + 1)], LSTM_DT, name=f"hall{k}")
                    nc.sync.dma_start(out=h[:, 0:BL], in_=h0T[128 * k:128 * (k + 1), :])
                    h_all.append(h)

                with tc.tile_pool(name="xw", bufs=1) as xwp:
                    xW = xwp.tile([128, 8 * BL * S], F32)  # (128, 32768)
                    xW4 = xW.rearrange("p (t m b) -> p t m b", m=8, b=BL)

                    # ---- Phase A ----
                    with tc.tile_pool(name="emb", bufs=2) as embp:
                        wih_sb = []
                        for k, (r0, r1) in enumerate(((0, 128), (128, 256), (256, EA))):
                            w = embp.tile([r1 - r0, G], F32, name=f"wih{k}",
                                          bufs=1, tag=f"wih{k}")
                            nc.sync.dma_start(out=w, in_=wihT[r0:r1, :])
                            wih_sb.append(w)
                        for nch in range(NCH):
                            ets = []
                            for k, (r0, r1) in enumerate(((0, 128), (128, 256), (256, EA))):
                                et = embp.tile([r1 - r0, CHT], F32,
                                               name=f"et{k}", tag=f"et{k}")
                                nc.sync.dma_start(
                                    out=et, in_=embT[r0:r1, CHT * nch:CHT * (nch + 1)])
                                ets.append(et)
                            for m in range(8):
                                pa = psp.tile([128, CHT], F32, name="pa", tag="A")
                                for k in range(3):
                                    nc.tensor.matmul(
                                        pa,
                                        wih_sb[k][:, 128 * m:128 * (m + 1)],
                                        ets[k],
                                        start=(k == 0), stop=(k == 2),
                                    )
                                tpc = CHT // BL  # steps per chunk
                                dst = xW4[:, tpc * nch:tpc * (nch + 1), m, :]
                                nc.scalar.copy(
                                    out=dst,
                                    in_=pa.rearrange("p (t b) -> p t b", b=BL),
                                )

                    # ---- Phase B: LSTM ----
                    with tc.tile_pool(name="step", bufs=3) as stp:
                        c_prev = stp.tile([128, 2 * BL], F32, name="cinit", tag="c", bufs=2)
                        nc.sync.dma_start(out=c_prev[:, 0:BL], in_=c0T[0:128, :])
                        nc.sync.dma_start(out=c_prev[:, BL:2 * BL], in_=c0T[128:256, :])
                        for t in range(S):
                            pg = psp.tile([128, 8 * BL], F32, name="pg", tag="G")
                            for m in range(8):
                                for k in range(2):
                                    nc.tensor.matmul(
                                        pg[:, BL * m:BL * (m + 1)],
                                        whh_sb[k][:, 128 * m:128 * (m + 1)],
                                        h_all[k][:, BL * t:BL * (t + 1)],
                                        start=(k == 0), stop=(k == 1),
                                    )
                            gates = stp.tile([128, 8 * BL], F32, name="gates", tag="gates")
                            nc.vector.tensor_tensor(
                                out=gates, in0=pg,
                                in1=xW[:, 8 * BL * t:8 * BL * (t + 1)], op=OP.add)
                            # col-groups (m*16+b): m 0,1 = g; 2,3 = i; 4,5 = f; 6,7 = o
                            tg = stp.tile([128, 2 * BL], F32, name="tg", tag="tg")
                            nc.scalar.activation(tg, gates[:, 0:2 * BL], ACTF.Tanh)
                            sio = stp.tile([128, 6 * BL], F32, name="sio", tag="sio")
                            nc.scalar.activation(
                                sio, gates[:, 2 * BL:8 * BL], ACTF.Sigmoid)
                            fc = stp.tile([128, 2 * BL], F32, name="fc", tag="fc")
                            nc.vector.tensor_tensor(
                                out=fc, in0=sio[:, 2 * BL:4 * BL], in1=c_prev, op=OP.mult)
                            ig = stp.tile([128, 2 * BL], F32, name="ig", tag="ig")
                            nc.vector.tensor_tensor(
                                out=ig, in0=sio[:, 0:2 * BL], in1=tg, op=OP.mult)
                            c_new = stp.tile([128, 2 * BL], F32, name="cnew", tag="c", bufs=2)
                            nc.vector.tensor_tensor(out=c_new, in0=fc, in1=ig, op=OP.add)
                            th = stp.tile([128, 2 * BL], F32, name="th", tag="th")
                            nc.scalar.activation(th, c_new, ACTF.Tanh)
                            for k in range(2):
                                nc.vector.tensor_tensor(
                                    out=h_all[k][:, BL * (t + 1):BL * (t + 2)],
                                    in0=sio[:, (4 + k) * BL:(5 + k) * BL],
                                    in1=th[:, BL * k:BL * (k + 1)],
                                    op=OP.mult,
                                )
                            c_prev = c_new

                # ---- Phase C: tag-space partials ----
                with tc.tile_pool(name="scanc", bufs=1) as scp:
                    ownP = scp.tile([128, 32 * T], F32)
                    for tt in range(32):
                        pc = psp.tile([128, T], F32, name="pc", tag="C")
                        for k in range(2):
                            nc.tensor.matmul(
                                pc,
                                h_all[k][:, BL + 128 * tt:BL + 128 * (tt + 1)],
                                WoT_sb[k],
                                start=(k == 0), stop=(k == 1),
                            )
                        nc.vector.tensor_copy(ownP[:, T * tt:T * (tt + 1)], pc)
                    nc.sync.dma_start(out=ownP_o[:, :], in_=ownP)

    _split_multiwaits(nc)
    return nc


def build_l2():
    nc = bass.Bass()

    sfA = nc.dram_tensor("sfA", [BL, S * T], F32, kind="ExternalInput")
    sfB = nc.dram_tensor("sfB", [BL, S * T], F32, kind="ExternalInput")
    transM = nc.dram_tensor("transM", [BL, T * T], F32, kind="ExternalInput")
    initv = nc.dram_tensor("initv", [BL, T], F32, kind="ExternalInput")
    iota2 = nc.dram_tensor("iota2", [BL, S * T], F32, kind="ExternalInput")
    path_o = nc.dram_tensor("path_o", [BL, S], F32, kind="ExternalOutput")
    score_o = nc.dram_tensor("score_o", [BL, 1], F32, kind="ExternalOutput")

    with TileContext(nc) as tc:
        with tc.tile_pool(name="dram", bufs=1, space="DRAM") as dramp, \
             tc.tile_pool(name="sc", bufs=1) as scp:
            cin2 = dramp.tile([2 * BL, S * T], F32)
            cout2 = dramp.tile([4 * BL, S * T], F32)

            transM_sb = scp.tile([BL, T * T], F32)
            nc.sync.dma_start(out=transM_sb, in_=transM[:, :])
            initv_sb = scp.tile([BL, T], F32)
            nc.sync.dma_start(out=initv_sb, in_=initv[:, :])
            iota2_sb = scp.tile([BL, S * T], F32)
            nc.sync.dma_start(out=iota2_sb, in_=iota2[:, :])
            sfA_sb = scp.tile([BL, S * T], F32)
            nc.sync.dma_start(out=sfA_sb, in_=sfA[:, :])
            sfB_sb = scp.tile([BL, S * T], F32)
            nc.sync.dma_start(out=sfB_sb, in_=sfB[:, :])
            sfeats = scp.tile([BL, S * T], F32)
            nc.vector.tensor_tensor(out=sfeats, in0=sfA_sb, in1=sfB_sb, op=OP.add)

            pre_all = scp.tile([BL, S * T], F32)
            uv_all = scp.tile([BL, S * T], F32)
            tM3 = transM_sb.rearrange("b (n p) -> b n p", n=T)
            with tc.tile_pool(name="scand", bufs=2) as sdp:
                prev = initv_sb
                for t in range(S):
                    ntv = sdp.tile([BL, T * T], F32, name="ntv", tag="ntv")
                    nc.vector.tensor_tensor(
                        out=ntv.rearrange("b (n p) -> b n p", n=T),
                        in0=prev.unsqueeze(1).broadcast_to([BL, T, T]),
                        in1=tM3, op=OP.add)
                    nc.vector.tensor_reduce(
                        out=pre_all[:, T * t:T * (t + 1)],
                        in_=ntv.rearrange("b (n p) -> b n p", n=T),
                        axis=AX.X, op=OP.max)
                    nc.vector.tensor_tensor(
                        out=uv_all[:, T * t:T * (t + 1)],
                        in0=pre_all[:, T * t:T * (t + 1)],
                        in1=sfeats[:, T * t:T * (t + 1)], op=OP.add)
                    prev = uv_all[:, T * t:T * (t + 1)]
            nc.gpsimd.dma_start(out=cin2[0:BL, :], in_=pre_all)
            nc.gpsimd.dma_start(out=cin2[BL:2 * BL, :], in_=uv_all)
            nc.gpsimd.collective_compute(
                "AllGather", OP.bypass,
                ins=[cin2.opt()], outs=[cout2.opt()],
                replica_groups=[[0, 4], [1, 5], [2, 6], [3, 7]],
            )

            def r3(ap):
                return ap.rearrange("b (s j) -> b s j", j=T)

            pr0 = scp.tile([BL, S * T], F32)
            nc.gpsimd.dma_start(out=pr0, in_=cout2[0:BL, :])
            uv1 = scp.tile([BL, S * T], F32)
            nc.gpsimd.dma_start(out=uv1, in_=cout2[3 * BL:4 * BL, :])
            tot = scp.tile([BL, S * T], F32)
            nc.vector.tensor_tensor(
                out=r3(tot), in0=r3(pr0), in1=r3(uv1)[:, ::-1, :], op=OP.add)
            vm = scp.tile([BL, S], F32)
            nc.vector.tensor_reduce(out=vm, in_=r3(tot), axis=AX.X, op=OP.max)
            msk = scp.tile([BL, S * T], F32)
            nc.vector.tensor_tensor(
                out=r3(msk), in0=r3(tot),
                in1=vm.unsqueeze(2).broadcast_to([BL, S, T]), op=OP.is_equal)
            prem = scp.tile([BL, S * T], F32)
            nc.vector.tensor_tensor(out=prem, in0=msk, in1=iota2_sb, op=OP.mult)
            pm = scp.tile([BL, S], F32)
            nc.vector.tensor_reduce(out=pm, in_=r3(prem), axis=AX.X, op=OP.min)
            path16 = scp.tile([BL, S], F32)
            nc.vector.tensor_scalar(
                out=path16, in0=pm, scalar1=-NEG, scalar2=None, op0=OP.add)
            nc.sync.dma_start(out=path_o[:, :], in_=path16)
            nc.sync.dma_start(out=score_o[:, :], in_=vm[:, 0:1])

    _split_multiwaits(nc)
    return nc


_NC_CACHE = {}
TRACE = False
LAST_EXEC_NS = []


def _get(name, builder):
    if name not in _NC_CACHE:
        _NC_CACHE[name] = builder()
    return _NC_CACHE[name]


def _np_dt(dt):
    import ml_dtypes
    return np.float32 if dt == F32 else ml_dtypes.bfloat16


def kernel(sentence, emb, wih_f, whh_f, bih_f, bhh_f, wih_b, whh_b, bih_b,
           bhh_b, W_out, b_out, transitions, h0, c0):
    sentence = np.asarray(sentence)
    emb = np.asarray(emb, np.float32)
    W_out = np.asarray(W_out, np.float32)
    b_out = np.asarray(b_out, np.float32)
    transitions = np.asarray(transitions, np.float32)
    h0 = np.asarray(h0, np.float32)
    c0 = np.asarray(c0, np.float32)
    wih_f, whh_f = np.asarray(wih_f, np.float32), np.asarray(whh_f, np.float32)
    wih_b, whh_b = np.asarray(wih_b, np.float32), np.asarray(whh_b, np.float32)
    bias_f = np.asarray(bih_f, np.float32) + np.asarray(bhh_f, np.float32)
    bias_b = np.asarray(bih_b, np.float32) + np.asarray(bhh_b, np.float32)

    # torch-style .view(S, B, -1) reinterpretation of the (B, S, E) gather
    embeds = emb[sentence.reshape(-1)].reshape(S, B, E)

    # gate reorder i,f,g,o -> g,i,f,o (PSUM col groups [g|i|f|o])
    perm = np.r_[2 * H2:3 * H2, 0:H2, H2:2 * H2, 3 * H2:4 * H2]
    ldt = _np_dt(LSTM_DT)

    in1 = []
    for c in range(NCORES):
        bwd = c >= 4
        b0 = BL * (c % 4)
        wih = (wih_b if bwd else wih_f)[perm]
        whh = (whh_b if bwd else whh_f)[perm]
        bias = (bias_b if bwd else bias_f)[perm]
        sl = embeds[:, b0:b0 + BL, :]
        if bwd:
            sl = sl[::-1]
        embT = np.concatenate(
            [sl.reshape(NTOK, E).T, np.ones((1, NTOK), np.float32)], axis=0)
        wihT = np.concatenate([wih.T, bias[None, :]], axis=0)
        d = 1 if bwd else 0
        Wo = (W_out[:, H2:] if bwd else W_out[:, :H2]).T
        in1.append({
            "embT": np.ascontiguousarray(embT, np.float32),
            "wihT": np.ascontiguousarray(wihT, np.float32),
            "whhT": np.ascontiguousarray(whh.T).astype(ldt),
            "h0T": np.ascontiguousarray(h0[d, b0:b0 + BL].T).astype(ldt),
            "c0T": np.ascontiguousarray(c0[d, b0:b0 + BL].T).astype(np.float32),
            "WoT": np.ascontiguousarray(Wo).astype(ldt),
        })

    nc1 = _get("l1", build_l1)
    res1 = run_bass_kernel_spmd(nc1, in1, core_ids=list(range(NCORES)))
    if TRACE:
        import time as _t
        t0 = _t.time()
        res1 = run_bass_kernel_spmd(nc1, in1, core_ids=list(range(NCORES)))
        LAST_EXEC_NS.append(("l1_wall", int((_t.time() - t0) * 1e9)))

    # ownP[c][16*(t%8)+bl, (t//8)*T+j]: partial for processing-token (t, bl)
    # fwd core k: token (t,bl) = h at (seq t, batch 16k+bl)
    # bwd core 4+k: token (t,bl) = h at (seq 255-t, batch 16k+bl)
    # -> featP[d, q, ob, j]: partial at seq q, orig batch ob, direction d
    featP = np.zeros((2, S, B, T), np.float32)
    for c in range(NCORES):
        P = res1.results[c]["ownP_o"].reshape(8, BL, 32, T)  # (u, bl, tt, j)
        P = P.transpose(2, 0, 1, 3).reshape(S, BL, T)        # t-major
        d, k = c // 4, c % 4
        if d == 1:
            P = P[::-1]
        featP[d, :, BL * k:BL * (k + 1), :] = P

    # Viterbi for batch b at step s consumes token (4b + s//64, s%64)
    svec = np.arange(S)
    in2 = []
    for c in range(NCORES):
        bwd = c >= 4
        b0 = BL * (c % 4)
        vb = np.arange(b0, b0 + BL)
        q = 4 * vb[:, None] + svec[None, :] // B     # (BL, S)
        ob = np.broadcast_to(svec[None, :] % B, q.shape)
        sfA = featP[0, q, ob, :]                     # (BL, S, T) fwd partial
        sfB = featP[1, q, ob, :]
        if bwd:
            sfA, sfB = sfA[:, ::-1], sfB[:, ::-1]
        tM = (transitions.T if bwd else transitions) + b_out[:, None]
        init = np.full(T, -10000.0, np.float32)
        init[STOP_IDX if bwd else START_IDX] = 0.0
        in2.append({
            "sfA": np.ascontiguousarray(sfA.reshape(BL, S * T), np.float32),
            "sfB": np.ascontiguousarray(sfB.reshape(BL, S * T), np.float32),
            "transM": np.tile(tM.reshape(1, T * T), (BL, 1)).astype(np.float32),
            "initv": np.tile(init[None, :], (BL, 1)),
            "iota2": np.tile((np.arange(T, dtype=np.float32) + NEG), (BL, S)),
        })

    nc2 = _get("l2", build_l2)
    res2 = run_bass_kernel_spmd(nc2, in2, core_ids=list(range(NCORES)))
    if TRACE:
        import time as _t
        t0 = _t.time()
        res2 = run_bass_kernel_spmd(nc2, in2, core_ids=list(range(NCORES)))
        LAST_EXEC_NS.append(("l2_wall", int((_t.time() - t0) * 1e9)))

    path = np.zeros((B, S), np.int32)
    score = np.zeros((B, 1), np.float32)
    for c in range(4):
        b0 = BL * c
        path[b0:b0 + BL] = np.rint(res2.results[c]["path_o"]).astype(np.int32)
        score[b0:b0 + BL] = res2.results[c]["score_o"]
    return score, path
